# revision 27
# baseline (speedup 1.0000x reference)
"""AttentionBlock (GroupNorm + single-head attention + proj + residual) on 8 TRN2
NeuronCores.

Reference computation (B=16, C=512, H=W=32, N=H*W=1024, 32 groups):
    h   = group_norm(x, gamma, beta)                      # [B,C,H,W]
    qkv = conv1x1(h, w_qkv) + b_qkv                       # [B,3C,H,W]
    s   = q^T k / sqrt(C); a = softmax(s, axis=-1)        # [B,N,N]
    o   = v @ a^T; out = x + conv1x1(o, w_proj) + b_proj  # [B,C,H,W]

Sharding: pure data-parallel over batch. B=16 -> 2 batch elements per core,
weights replicated, no collectives.

v3 structure (per batch element, all [partition, free]):
    x         : [c, n] bf16, per-chunk tiles (first chunk as 512-halves so
                GroupNorm stats start the moment data lands)
    h         : [c, n] fp8, GroupNorm pipelined per chunk-PAIR: the chain
                (group-reduce matmul + rstd + scale/shift) runs per pair so
                h(pair0) unblocks the first g matmuls ~4us earlier
    rstd      : exp(-0.5*ln(var+eps)) on ACT — the whole kernel then only
                uses {Exp, Ln, Identity, Square, Copy}, all members of the
                single `natural_log_exp_and_others` ACT table set: ZERO
                mid-kernel ACT table switches (the sqrt/exp thrash cost the
                v2 kernel ~5 ACT_TABLE_LOADs at ~1.3-2.7us each)
    stats     : split across engines: 2 chunks via DVE bn_stats, 2 chunks
                via ACT activation accum (Identity/Square with scale 1/1024
                and 1/32 directly produce mean and E[x^2])
    g = M h   : M = (Wq^T Wk)*WS host-precomputed (replaces q AND k projs)
    vT        : [n, c] fp8 via swapped matmul; carries (w_proj @ w_v)*WS
    eT=exp(.) : [j, i] fp8 = exp(s)/16 (dodges fp8e4 saturation)
    denom     : ones-matmul over j -> reciprocal (applied post-AV)
    av        : [c, i] = vT^T @ eT, scaled by recip, +x residual
Phase order: head(0), g/vt(0), sc(0) [b1's GN overlaps], g/vt(1), sc(1),
av(0), av(1) — b1's exp stream fully precedes av(1) so the final av matmuls
never stall on ACT, and the b0 exp stream overlaps g/vt(1) matmuls.
HAM warm-up: real-sized junk matmuls (x^T x) through the head window keep
the PE clock-gate at 8/8 so the first real matmuls run at 2.4GHz.
PSUM: P1 = 4x [128,512] (junk, gn-stat reduces, g/vt half-tiles, last av
tile halves), P2 = 2x [128,1024] (scores/denom/av tiles) = 8 banks.

b_eff = w_proj @ b_v + b_proj rides x from the host (GroupNorm is shift-
invariant and softmax rows sum to 1). All big matmuls fp8 DoubleRow
(weights pre-scaled by WS=8); PSUM f32; GN statistics in f32.
"""

import sys

for _p in ("/opt/trn_rl_repo", "/opt/pypackages"):
    if _p not in sys.path:
        sys.path.append(_p)

import numpy as np
import ml_dtypes

import concourse.bass as bass
import concourse.bacc as bacc
import concourse.tile as tile
from concourse import mybir

AF = mybir.ActivationFunctionType
OP = mybir.AluOpType
F32 = mybir.dt.float32
BF16 = mybir.dt.bfloat16
FP8 = mybir.dt.float8e4
DR = mybir.MatmulPerfMode.DoubleRow
LN16 = 2.772588722239781  # eT stored as exp(s)/16 in fp8e4

N_CORES = 8
B, C, H, W = 16, 512, 32, 32
N = H * W               # 1024 pixels
BPC = B // N_CORES      # batch elements per core = 2
GROUPS = 32
EPS = 1e-5
KT = C // 128           # 4 contraction chunks over channels
NT = N // 128           # 8 chunks over pixels
SCALE = 1.0 / np.sqrt(np.float32(C))
WS = 8.0          # fp8 weight pre-scale


def _force_single_act_table():
    """Constrain the ACT table-set chooser to `natural_log_exp_and_others`
    (contains Exp, Ln, Identity, Square, Copy — every func this kernel uses)
    so walrus cannot thrash between per-anchor sets (each switch ~2.7us on
    the ACT critical path). List order and length are preserved so the
    emitted act_func_set_id indices stay valid."""
    if getattr(bacc, "_act_tables_forced", False):
        return
    orig = bacc.get_activation_tables

    def patched(arch):
        tabs = orig(arch)
        return {
            name: (fns if name == "natural_log_exp_and_others" else set())
            for name, fns in tabs.items()
        }

    patched.__wrapped__ = orig
    bacc.get_activation_tables = patched
    bacc._act_tables_forced = True


def build_nc():
    _force_single_act_table()
    nc = bacc.Bacc("TRN2", target_bir_lowering=False)

    x_ext = nc.declare_dram_parameter("x", [BPC, C, N], BF16, isOutput=False)
    wm_ext = nc.declare_dram_parameter("wm", [C, C], FP8, isOutput=False)
    wu_ext = nc.declare_dram_parameter("wu", [C, C], FP8, isOutput=False)
    gmat_ext = nc.declare_dram_parameter("gmat", [128, 128], BF16, isOutput=False)
    ones_ext = nc.declare_dram_parameter("ones", [128, 256], FP8, isOutput=False)
    out_ext = nc.declare_dram_parameter("out", [BPC, C, N], BF16, isOutput=True)

    with tile.TileContext(nc) as tc:
        with (
            tc.tile_pool(name="wpool", bufs=1) as wpool,
            tc.tile_pool(name="xpool", bufs=1) as xpool,
            tc.tile_pool(name="hpool", bufs=1) as hpool,
            tc.tile_pool(name="gpool", bufs=1) as gpool,
            tc.tile_pool(name="vepool", bufs=1) as vepool,
            tc.tile_pool(name="avpool", bufs=1) as avpool,
            tc.tile_pool(name="opool", bufs=4) as opool,
            tc.tile_pool(name="stpool", bufs=2) as stpool,
            tc.tile_pool(name="ps1", bufs=4, space="PSUM") as ps1,   # [128,512]
            tc.tile_pool(name="ps2", bufs=2, space="PSUM") as ps2,   # [128,1024]
        ):
            eps_sb = wpool.tile([128, 1], F32)
            nc.vector.memset(eps_sb, EPS)
            nln16_sb = wpool.tile([128, 1], F32)
            nc.vector.memset(nln16_sb, -LN16)

            # ---- input DMAs, first-use order, alternating queues ----
            xr = [x_ext[b].rearrange("(ko p) n -> p ko n", p=128) for b in range(BPC)]
            x0c0h = [
                xpool.tile([128, 512], BF16, name=f"x0c0h{i}", bufs=1) for i in range(2)
            ]
            x0_chunks = [None] + [
                xpool.tile([128, N], BF16, name=f"x0_{ki}", bufs=1)
                for ki in range(1, KT)
            ]
            nc.sync.dma_start(out=x0c0h[0], in_=xr[0][:, 0, 0:512])
            nc.gpsimd.dma_start(out=x0c0h[1], in_=xr[0][:, 0, 512:1024])
            nc.sync.dma_start(out=x0_chunks[1], in_=xr[0][:, 1, :])
            nc.gpsimd.dma_start(out=x0_chunks[2], in_=xr[0][:, 2, :])
            nc.sync.dma_start(out=x0_chunks[3], in_=xr[0][:, 3, :])
            gmat = wpool.tile([128, 128], BF16)
            nc.gpsimd.dma_start(out=gmat, in_=gmat_ext[:])
            wm = wpool.tile([128, KT, C], FP8)
            nc.sync.dma_start(out=wm, in_=wm_ext[:].rearrange("(ko p) f -> p ko f", p=128))
            wu = wpool.tile([128, KT, C], FP8)
            nc.gpsimd.dma_start(out=wu, in_=wu_ext[:].rearrange("(ko p) f -> p ko f", p=128))
            ones = wpool.tile([128, 256], FP8)
            nc.sync.dma_start(out=ones, in_=ones_ext[:])

            x1_pairs = [
                xpool.tile([128, 2, N], BF16, name=f"x1_{pr}", bufs=1) for pr in range(2)
            ]

            def x_chunk(b, ki):
                if b == 0:
                    if ki == 0:
                        return None  # halves; handled by callers
                    return x0_chunks[ki]
                return x1_pairs[ki // 2][:, ki % 2, :]

            # dummy ln: forces the single (ln+exp) ACT table load at t~7us
            warm = stpool.tile([128, 1], F32, name="warm")
            nc.scalar.activation(out=warm, in_=eps_sb, func=AF.Ln)

            h_prs = [
                [hpool.tile([128, 2, N], FP8, name=f"h_{b}_{pr}", bufs=1) for pr in range(2)]
                for b in range(BPC)
            ]
            g_sbs = [gpool.tile([128, KT, N], FP8, name=f"g_{b}", bufs=1) for b in range(BPC)]
            vTs = [vepool.tile([128, NT, C], FP8, name=f"vT_{b}", bufs=1) for b in range(BPC)]
            eTs = [vepool.tile([128, NT, N], FP8, name=f"eT_{b}", bufs=1) for b in range(BPC)]
            recips = [avpool.tile([128, N], F32, name=f"recip_{b}", bufs=1) for b in range(BPC)]
            # ACT accum scratch (full-width mandatory output of accum ops)
            scr = wpool.tile([128, N], F32)

            # per-batch stats: mv[:, ki, 0]=mean, mv[:, ki, 1]=E[x^2] (or var
            # pre-conversion for bn_stats chunks)
            mvs = [stpool.tile([128, KT, 2], F32, name=f"mv{b}", bufs=1) for b in range(BPC)]
            scls = [stpool.tile([128, KT], F32, name=f"scl{b}", bufs=1) for b in range(BPC)]
            sfts = [stpool.tile([128, KT], F32, name=f"sft{b}", bufs=1) for b in range(BPC)]

            def bn_chunk(b, ki, in0, in1):
                # DVE bn_stats path -> mv[:, ki, :] = (mean, var)
                mv = mvs[b]
                stats = stpool.tile([128, 2, 6], F32, name="stats")
                nc.vector.bn_stats(out=stats[:, 0, :], in_=in0)
                nc.vector.bn_stats(out=stats[:, 1, :], in_=in1)
                nc.vector.bn_aggr(out=mv[:, ki, :], in_=stats)

            def act_sum(b, ki, xc):
                # ACT accum: mean via Identity with scale 1/N
                nc.scalar.activation(
                    out=scr, in_=xc, func=AF.Identity, scale=1.0 / N,
                    accum_out=mvs[b][:, ki, 0:1],
                )

            def act_sq(b, ki, xc):
                # ACT accum: E[x^2] via Square with scale 1/32
                nc.scalar.activation(
                    out=scr, in_=xc, func=AF.Square, scale=1.0 / 32.0,
                    accum_out=mvs[b][:, ki, 1:2],
                )

            def e2_convert(b, ki):
                # var -> E[x^2] in place
                mv = mvs[b]
                msq = stpool.tile([128, 1], F32, name="msq")
                nc.vector.tensor_tensor(msq, mv[:, ki, 0:1], mv[:, ki, 0:1], OP.mult)
                nc.vector.tensor_tensor(mv[:, ki, 1:2], mv[:, ki, 1:2], msq, OP.add)

            def chain_pair(b, p):
                # group-reduce (mean, E2) for chunks 2p,2p+1 via gmat matmul,
                # then scl/sft for the pair.
                mv = mvs[b]
                mv_bf = stpool.tile([128, 4], BF16, name=f"mvbf{b}{p}", bufs=1)
                nc.vector.tensor_copy(
                    out=mv_bf, in_=mv[:, 2 * p : 2 * p + 2, :].rearrange("p a b -> p (a b)")
                )
                gn_ps = ps1.tile([128, 512], F32, name=f"gnps{b}{p}", tag="ps1")
                nc.tensor.matmul(gn_ps[:, 0:4], lhsT=gmat, rhs=mv_bf, start=True, stop=True)
                gs = stpool.tile([128, 4], F32, name=f"gs{b}{p}", bufs=1)
                nc.vector.tensor_copy(out=gs, in_=gn_ps[:, 0:4])
                gmean = gs[:, 0:4:2]
                gex2 = gs[:, 1:4:2]
                gvar = stpool.tile([128, 2], F32, name=f"gvar{b}{p}", bufs=1)
                nc.vector.tensor_tensor(gvar, gmean, gmean, OP.mult)
                nc.vector.tensor_tensor(gvar, gex2, gvar, OP.subtract)
                lnv = stpool.tile([128, 2], F32, name=f"lnv{b}{p}", bufs=1)
                nc.scalar.activation(out=lnv, in_=gvar, func=AF.Ln, bias=eps_sb)
                # rstd == scl (gamma==1, beta==0 guarded on host)
                scl_p = scls[b][:, 2 * p : 2 * p + 2]
                sft_p = sfts[b][:, 2 * p : 2 * p + 2]
                rstd = stpool.tile([128, 2], F32, name=f"rstd{b}{p}", bufs=1)
                nc.scalar.activation(out=rstd, in_=lnv, func=AF.Exp, scale=-0.5)
                nc.vector.tensor_copy(out=scl_p, in_=rstd)
                mscl = stpool.tile([128, 2], F32, name=f"mscl{b}{p}", bufs=1)
                nc.vector.tensor_tensor(mscl, gmean, scl_p, OP.mult)
                nc.vector.tensor_scalar_mul(sft_p, mscl, -1.0)
                return mv_bf

            def h_op(b, ki, eng, src, dst):
                eng.tensor_scalar(
                    out=dst, in0=src,
                    scalar1=scls[b][:, ki : ki + 1], scalar2=sfts[b][:, ki : ki + 1],
                    op0=OP.mult, op1=OP.add,
                )

            # ================= b0 head =================
            bn_chunk(0, 0, x0c0h[0], x0c0h[1])
            act_sum(0, 1, x0_chunks[1])
            act_sq(0, 1, x0_chunks[1])
            e2_convert(0, 0)
            mvbf00 = chain_pair(0, 0)
            act_sum(0, 2, x0_chunks[2])
            act_sq(0, 2, x0_chunks[2])
            # gate b1's x DMA on an early b0-chain token: late enough that
            # the scheduler cannot hoist b1's work over b0's critical chain,
            # early enough that x1 lands while ACT still has stats headroom
            for pr in range(2):
                nc.gpsimd.tensor_scalar_add(
                    x1_pairs[pr][:, 0, 0:1], mvbf00[:, 0:1], 0.0
                )
            nc.sync.dma_start(out=x1_pairs[0], in_=xr[1][:, 0:2, :])
            nc.gpsimd.dma_start(out=x1_pairs[1], in_=xr[1][:, 2:4, :])
            act_sum(0, 3, x0_chunks[3])
            act_sq(0, 3, x0_chunks[3])
            chain_pair(0, 1)

            # h(0): ki0 (halves) + ki2 on DVE, ki1/ki3 on GPSIMD
            h_op(0, 0, nc.vector, x0c0h[0], h_prs[0][0][:, 0, 0:512])
            h_op(0, 0, nc.vector, x0c0h[1], h_prs[0][0][:, 0, 512:1024])
            h_op(0, 1, nc.gpsimd, x0_chunks[1], h_prs[0][0][:, 1, :])
            h_op(0, 2, nc.vector, x0_chunks[2], h_prs[0][1][:, 0, :])
            h_op(0, 3, nc.gpsimd, x0_chunks[3], h_prs[0][1][:, 1, :])

            # ================= g(0) / vt(0): half-tiles in PS1 =================
            # g half-tile (oi, ni): [128,512] accumulating kk0,kk1
            # vt half-tile (nn, sub) likewise
            def g_mm(b, oi, ni, kk, ps):
                nc.tensor.matmul(
                    ps,
                    lhsT=wm[:, 2 * kk : 2 * kk + 2, oi * 128 : (oi + 1) * 128],
                    rhs=h_prs[b][kk][:, :, ni * 512 : (ni + 1) * 512],
                    start=(kk == 0), stop=(kk == 1), perf_mode=DR,
                )

            def vt_mm(b, ni, kk, ps):
                nc.tensor.matmul(
                    ps,
                    lhsT=h_prs[b][kk][:, :, ni * 128 : (ni + 1) * 128],
                    rhs=wu[:, 2 * kk : 2 * kk + 2, :],
                    start=(kk == 0), stop=(kk == 1), perf_mode=DR,
                )

            def g_drain(b, oi, ni, ps, eng):
                dst = g_sbs[b][:, oi, ni * 512 : (ni + 1) * 512]
                if eng is nc.scalar:
                    nc.scalar.activation(out=dst, in_=ps, func=AF.Identity)
                else:
                    eng.tensor_copy(out=dst, in_=ps)

            def vt_drain(b, ni, ps, eng):
                dst = vTs[b][:, ni, :]
                if eng is nc.scalar:
                    nc.scalar.activation(out=dst, in_=ps, func=AF.Identity)
                else:
                    eng.tensor_copy(out=dst, in_=ps)

            def emit_g(b, interleave_kk, acts):
                # 8 g half-tiles (oi, ni), 2 MMs each (kk0, kk1); `acts` of
                # the drains go to ACT, rest DVE
                keys = [(oi, ni) for oi in range(KT) for ni in range(2)]
                if interleave_kk:
                    # kk0s of 3 tiles in flight so pair1's late h never
                    # bubbles the PE queue
                    pipe = []
                    for idx, (oi, ni) in enumerate(keys):
                        ps = ps1.tile([128, 512], F32, name="gvt", tag="ps1")
                        g_mm(b, oi, ni, 0, ps)
                        pipe.append((oi, ni, ps))
                        if idx >= 3:
                            o2, n2, ps2_ = pipe.pop(0)
                            g_mm(b, o2, n2, 1, ps2_)
                            g_drain(b, o2, n2, ps2_,
                                    nc.scalar if (o2 * 2 + n2) < acts else nc.vector)
                    while pipe:
                        o2, n2, ps2_ = pipe.pop(0)
                        g_mm(b, o2, n2, 1, ps2_)
                        g_drain(b, o2, n2, ps2_,
                                nc.scalar if (o2 * 2 + n2) < acts else nc.vector)
                else:
                    for idx, (oi, ni) in enumerate(keys):
                        ps = ps1.tile([128, 512], F32, name="gvt", tag="ps1")
                        g_mm(b, oi, ni, 0, ps)
                        g_mm(b, oi, ni, 1, ps)
                        g_drain(b, oi, ni, ps,
                                nc.scalar if idx % 2 == 0 and idx < 2 * acts else nc.vector)

            def emit_vt(b, acts):
                for ni in range(NT):
                    ps = ps1.tile([128, 512], F32, name="gvt", tag="ps1")
                    vt_mm(b, ni, 0, ps)
                    vt_mm(b, ni, 1, ps)
                    vt_drain(b, ni, ps,
                             nc.scalar if ni % 2 == 0 and ni < 2 * acts else nc.vector)

            # b1 stats ride the otherwise-idle ACT window between b0's
            # chain and the b0 exp stream (accum -> (mean, E2) directly)
            for ki in range(KT):
                act_sum(1, ki, x_chunk(1, ki))
                act_sq(1, ki, x_chunk(1, ki))

            emit_g(0, interleave_kk=True, acts=2)
            emit_vt(0, acts=0)

            # ================= scores =================
            def emit_scores(b, mid=None):
                # eT = exp(g^T h * SCALE/WS - ln16)  [j, i]; denominators in
                # PS1 halves; recip on DVE. `mid` emits extra work after ji4.
                h_pr = h_prs[b]
                g_sb = g_sbs[b]
                eT = eTs[b]
                den_h = [ps1.tile([128, 512], F32, name=f"den{b}{i}", tag="ps1") for i in range(2)]

                def denom_mm(jj):
                    for ni in range(2):
                        nc.tensor.matmul(
                            den_h[ni],
                            lhsT=ones.rearrange("p (two f) -> p two f", two=2),
                            rhs=eT[:, 2 * jj : 2 * jj + 2, ni * 512 : (ni + 1) * 512],
                            start=(jj == 0), stop=(jj == NT // 2 - 1),
                            perf_mode=DR,
                        )

                for ji in range(NT):
                    ps = ps2.tile([128, N], F32, name="scps", tag="ps2")
                    for kk in range(2):
                        for ni in range(2):
                            nc.tensor.matmul(
                                ps[:, ni * 512 : (ni + 1) * 512],
                                lhsT=g_sb[:, 2 * kk : 2 * kk + 2, ji * 128 : (ji + 1) * 128],
                                rhs=h_pr[kk][:, :, ni * 512 : (ni + 1) * 512],
                                start=(kk == 0), stop=(kk == 1),
                                perf_mode=DR,
                            )
                    nc.scalar.activation(
                        out=eT[:, ji, :], in_=ps, func=AF.Exp,
                        bias=nln16_sb, scale=float(SCALE / WS),
                    )
                    if ji >= 3 and ji % 2 == 1:
                        denom_mm((ji - 3) // 2)
                    # mid at ji==2: early enough that b1's h is ready before
                    # g(1), late enough that the tiny gstat matmuls are
                    # data-ready when the in-order PE queue reaches them
                    if ji == 2 and mid is not None:
                        mid()
                denom_mm(NT // 2 - 1)
                recip = recips[b]
                nc.vector.reciprocal_approx_fast(out=recip[:, 0:512], in_=den_h[0])
                nc.vector.reciprocal_approx_fast(out=recip[:, 512:1024], in_=den_h[1])

            def b1_mid():
                # b1's GN chain + h, overlapped with b0's exp stream (the
                # ln/rstd ACT ops share the resident table set)
                chain_pair(1, 0)
                chain_pair(1, 1)
                x1c = [x_chunk(1, ki) for ki in range(KT)]
                h_op(1, 0, nc.vector, x1c[0], h_prs[1][0][:, 0, :])
                h_op(1, 1, nc.gpsimd, x1c[1], h_prs[1][0][:, 1, :])
                h_op(1, 2, nc.vector, x1c[2], h_prs[1][1][:, 0, :])
                h_op(1, 3, nc.gpsimd, x1c[3], h_prs[1][1][:, 1, :])

            emit_scores(0, mid=b1_mid)
            emit_g(1, interleave_kk=False, acts=2)
            emit_vt(1, acts=2)
            emit_scores(1)

            # ================= av =================
            def emit_av(b):
                vT = vTs[b]
                eT = eTs[b]
                recip = recips[b]
                o_r = out_ext[b].rearrange("(ko p) n -> p ko n", p=128)
                for ci in range(KT):
                    last = b == 1 and ci == KT - 1
                    if not last:
                        ps = ps2.tile([128, N], F32, name="avps", tag="ps2")
                        for jj in range(NT // 2):
                            for ni in range(2):
                                nc.tensor.matmul(
                                    ps[:, ni * 512 : (ni + 1) * 512],
                                    lhsT=vT[:, 2 * jj : 2 * jj + 2, ci * 128 : (ci + 1) * 128],
                                    rhs=eT[:, 2 * jj : 2 * jj + 2, ni * 512 : (ni + 1) * 512],
                                    start=(jj == 0), stop=(jj == NT // 2 - 1),
                                    perf_mode=DR,
                                )
                        t1 = opool.tile([128, N], BF16, name="t1_sb")
                        o_sb = opool.tile([128, N], BF16, name="o_sb")
                        nc.vector.tensor_tensor(t1, ps, recip, OP.mult)
                        # +x residual: gpsimd for early tiles (idle engine),
                        # DVE for late ones (gpsimd is slow, would drag tail)
                        add_eng = nc.gpsimd if ci < 2 else nc.vector
                        if b == 0 and ci == 0:
                            add_eng.tensor_tensor(o_sb[:, 0:512], t1[:, 0:512], x0c0h[0], OP.add)
                            add_eng.tensor_tensor(o_sb[:, 512:1024], t1[:, 512:1024], x0c0h[1], OP.add)
                        else:
                            add_eng.tensor_tensor(o_sb, t1, x_chunk(b, ci), OP.add)
                        eng = nc.sync if ci % 2 == 0 else nc.gpsimd
                        eng.dma_start(out=o_r[:, ci, :], in_=o_sb)
                    else:
                        # final tile: ni-major so each 512-half drains the
                        # moment its accumulation closes
                        ph = [ps1.tile([128, 512], F32, name=f"avl{i}", tag="ps1") for i in range(2)]
                        for ni in range(2):
                            for jj in range(NT // 2):
                                nc.tensor.matmul(
                                    ph[ni],
                                    lhsT=vT[:, 2 * jj : 2 * jj + 2, ci * 128 : (ci + 1) * 128],
                                    rhs=eT[:, 2 * jj : 2 * jj + 2, ni * 512 : (ni + 1) * 512],
                                    start=(jj == 0), stop=(jj == NT // 2 - 1),
                                    perf_mode=DR,
                                )
                            t1 = opool.tile([128, 512], BF16, name="t1h_sb")
                            o_sb = opool.tile([128, 512], BF16, name="oh_sb")
                            sl = slice(ni * 512, (ni + 1) * 512)
                            nc.vector.tensor_tensor(t1, ph[ni], recip[:, sl], OP.mult)
                            nc.vector.tensor_tensor(o_sb, t1, x_chunk(b, ci)[:, sl], OP.add)
                            eng = nc.gpsimd if ni == 0 else nc.sync
                            eng.dma_start(out=o_r[:, ci, sl], in_=o_sb)

            emit_av(0)
            emit_av(1)

    nc.compile()
    return nc


_NC_CACHE = None


def _get_nc():
    global _NC_CACHE
    if _NC_CACHE is None:
        _NC_CACHE = build_nc()
    return _NC_CACHE


def _prep_consts(gamma, beta, w_qkv, b_qkv, w_proj, b_proj):
    bf = ml_dtypes.bfloat16
    f8 = ml_dtypes.float8_e4m3
    w_q, w_k, w_v = w_qkv[0:C], w_qkv[C : 2 * C], w_qkv[2 * C : 3 * C]
    b_v = b_qkv[2 * C : 3 * C]
    m = w_q.astype(np.float64).T @ w_k.astype(np.float64)  # [C, C]
    wm = np.ascontiguousarray(m.T * WS).astype(f8)         # lhsT layout [b, a]
    u = w_proj.astype(np.float64) @ w_v.astype(np.float64)  # [C, C] proj-folded V
    wu = np.ascontiguousarray(u.T * WS).astype(f8)
    b_eff = w_proj.astype(np.float64) @ b_v.astype(np.float64) + b_proj
    gmat = (np.kron(np.eye(8, dtype=np.float32), np.ones((16, 16), np.float32)) / 16.0).astype(bf)
    # denominator lhsT: value WS compensates vT carrying a factor of WS
    ones = np.full((128, 256), WS, f8)
    return wm, wu, b_eff.astype(np.float32), gmat, ones


def make_in_maps(x, gamma, beta, w_qkv, b_qkv, w_proj, b_proj):
    bf = ml_dtypes.bfloat16
    x = np.asarray(x, np.float32)
    gamma = np.asarray(gamma, np.float32)
    beta = np.asarray(beta, np.float32)
    w_qkv = np.asarray(w_qkv, np.float32)
    b_qkv = np.asarray(b_qkv, np.float32)
    w_proj = np.asarray(w_proj, np.float32)
    b_proj = np.asarray(b_proj, np.float32)
    wm, wu, b_eff, gmat, ones = _prep_consts(
        gamma, beta, w_qkv, b_qkv, w_proj, b_proj
    )
    # b_eff rides the residual input: GroupNorm is invariant to a per-channel
    # shift (the mean absorbs it), and softmax rows sum to 1, so shipping
    # x + b_eff makes out = (x + b_eff) + U h A exactly the reference result.
    xr = np.ascontiguousarray(
        (x.reshape(B, C, N) + b_eff[None, :, None]).astype(bf)
    )
    return [
        {
            "x": xr[i * BPC : (i + 1) * BPC],
            "wm": wm,
            "wu": wu,
            "gmat": gmat,
            "ones": ones,
        }
        for i in range(N_CORES)
    ]


def _numpy_fallback(x, gamma, beta, w_qkv, b_qkv, w_proj, b_proj):
    # Exact reference implementation; only used when b_q is nonzero (the
    # device graph folds Wq^T Wk and drops the q-bias term, which is exact
    # for this model where b_qkv == 0).
    Bs, Cs, Hs, Ws_ = x.shape
    g = x.reshape(Bs, GROUPS, Cs // GROUPS, Hs, Ws_)
    mu = g.mean(axis=(2, 3, 4), keepdims=True)
    var = g.var(axis=(2, 3, 4), keepdims=True)
    g = (g - mu) / np.sqrt(var + EPS)
    h = g.reshape(Bs, Cs, Hs, Ws_) * gamma[None, :, None, None] + beta[None, :, None, None]
    hn = h.reshape(Bs, Cs, N)
    qkv = np.einsum("bcn,oc->bon", hn, w_qkv) + b_qkv[None, :, None]
    q, k, v = qkv[:, :Cs], qkv[:, Cs : 2 * Cs], qkv[:, 2 * Cs :]
    s = np.einsum("bci,bcj->bij", q, k) / np.sqrt(np.float32(Cs))
    s = s - s.max(axis=-1, keepdims=True)
    e = np.exp(s)
    a = e / e.sum(axis=-1, keepdims=True)
    o = np.einsum("bij,bcj->bci", a, v)
    o = np.einsum("bcn,oc->bon", o, w_proj) + b_proj[None, :, None]
    return (x + o.reshape(Bs, Cs, Hs, Ws_)).astype(np.float32)


def kernel(x, gamma, beta, w_qkv, b_qkv, w_proj, b_proj):
    from concourse.bass_utils import run_bass_kernel_spmd

    x = np.asarray(x, np.float32)
    b_qkv = np.asarray(b_qkv, np.float32)
    gamma_a = np.asarray(gamma, np.float32)
    beta_a = np.asarray(beta, np.float32)
    # the device graph hardcodes gamma==1 / beta==0 (true for this model);
    # exact numpy path otherwise
    if (
        np.abs(b_qkv[0:C]).max() > 1e-7
        or np.abs(gamma_a - 1.0).max() > 1e-7
        or np.abs(beta_a).max() > 1e-7
    ):
        return _numpy_fallback(
            x, np.asarray(gamma, np.float32), np.asarray(beta, np.float32),
            np.asarray(w_qkv, np.float32), b_qkv,
            np.asarray(w_proj, np.float32), np.asarray(b_proj, np.float32),
        )

    nc = _get_nc()
    in_maps = make_in_maps(x, gamma, beta, w_qkv, b_qkv, w_proj, b_proj)
    res = run_bass_kernel_spmd(nc, in_maps, core_ids=list(range(N_CORES)))
    out = np.concatenate([res.results[i]["out"] for i in range(N_CORES)], axis=0)
    return np.ascontiguousarray(out.reshape(B, C, H, W), dtype=np.float32)


# revision 28
# speedup vs baseline: 1.0264x; 1.0264x over previous
"""AttentionBlock (GroupNorm + single-head attention + proj + residual) on 8 TRN2
NeuronCores.

Reference computation (B=16, C=512, H=W=32, N=H*W=1024, 32 groups):
    h   = group_norm(x, gamma, beta)                      # [B,C,H,W]
    qkv = conv1x1(h, w_qkv) + b_qkv                       # [B,3C,H,W]
    s   = q^T k / sqrt(C); a = softmax(s, axis=-1)        # [B,N,N]
    o   = v @ a^T; out = x + conv1x1(o, w_proj) + b_proj  # [B,C,H,W]

Sharding: pure data-parallel over batch. B=16 -> 2 batch elements per core,
weights replicated, no collectives.

v3 structure (per batch element, all [partition, free]):
    x         : [c, n] bf16, per-chunk tiles (first chunk as 512-halves so
                GroupNorm stats start the moment data lands)
    h         : [c, n] fp8, GroupNorm pipelined per chunk-PAIR: the chain
                (group-reduce matmul + rstd + scale/shift) runs per pair so
                h(pair0) unblocks the first g matmuls ~4us earlier
    rstd      : exp(-0.5*ln(var+eps)) on ACT — the whole kernel then only
                uses {Exp, Ln, Identity, Square, Copy}, all members of the
                single `natural_log_exp_and_others` ACT table set: ZERO
                mid-kernel ACT table switches (the sqrt/exp thrash cost the
                v2 kernel ~5 ACT_TABLE_LOADs at ~1.3-2.7us each)
    stats     : split across engines: 2 chunks via DVE bn_stats, 2 chunks
                via ACT activation accum (Identity/Square with scale 1/1024
                and 1/32 directly produce mean and E[x^2])
    g = M h   : M = (Wq^T Wk)*WS host-precomputed (replaces q AND k projs)
    vT        : [n, c] fp8 via swapped matmul; carries (w_proj @ w_v)*WS
    eT=exp(.) : [j, i] fp8 = exp(s)/16 (dodges fp8e4 saturation)
    denom     : ones-matmul over j -> reciprocal (applied post-AV)
    av        : [c, i] = vT^T @ eT, scaled by recip, +x residual
Phase order: head(0), g/vt(0), sc(0) [b1's GN overlaps], g/vt(1), sc(1),
av(0), av(1) — b1's exp stream fully precedes av(1) so the final av matmuls
never stall on ACT, and the b0 exp stream overlaps g/vt(1) matmuls.
HAM warm-up: real-sized junk matmuls (x^T x) through the head window keep
the PE clock-gate at 8/8 so the first real matmuls run at 2.4GHz.
PSUM: P1 = 4x [128,512] (junk, gn-stat reduces, g/vt half-tiles, last av
tile halves), P2 = 2x [128,1024] (scores/denom/av tiles) = 8 banks.

b_eff = w_proj @ b_v + b_proj rides x from the host (GroupNorm is shift-
invariant and softmax rows sum to 1). All big matmuls fp8 DoubleRow
(weights pre-scaled by WS=8); PSUM f32; GN statistics in f32.
"""

import sys

for _p in ("/opt/trn_rl_repo", "/opt/pypackages"):
    if _p not in sys.path:
        sys.path.append(_p)

import numpy as np
import ml_dtypes

import concourse.bass as bass
import concourse.bacc as bacc
import concourse.tile as tile
from concourse import mybir

AF = mybir.ActivationFunctionType
OP = mybir.AluOpType
F32 = mybir.dt.float32
BF16 = mybir.dt.bfloat16
FP8 = mybir.dt.float8e4
DR = mybir.MatmulPerfMode.DoubleRow
LN16 = 2.772588722239781  # eT stored as exp(s)/16 in fp8e4

N_CORES = 8
B, C, H, W = 16, 512, 32, 32
N = H * W               # 1024 pixels
BPC = B // N_CORES      # batch elements per core = 2
GROUPS = 32
EPS = 1e-5
KT = C // 128           # 4 contraction chunks over channels
NT = N // 128           # 8 chunks over pixels
SCALE = 1.0 / np.sqrt(np.float32(C))
WS = 8.0          # fp8 weight pre-scale


def _force_single_act_table():
    """Constrain the ACT table-set chooser to `natural_log_exp_and_others`
    (contains Exp, Ln, Identity, Square, Copy — every func this kernel uses)
    so walrus cannot thrash between per-anchor sets (each switch ~2.7us on
    the ACT critical path). List order and length are preserved so the
    emitted act_func_set_id indices stay valid."""
    if getattr(bacc, "_act_tables_forced", False):
        return
    orig = bacc.get_activation_tables

    def patched(arch):
        tabs = orig(arch)
        return {
            name: (fns if name == "natural_log_exp_and_others" else set())
            for name, fns in tabs.items()
        }

    patched.__wrapped__ = orig
    bacc.get_activation_tables = patched
    bacc._act_tables_forced = True


def build_nc():
    _force_single_act_table()
    nc = bacc.Bacc("TRN2", target_bir_lowering=False)

    x_ext = nc.declare_dram_parameter("x", [BPC, C, N], BF16, isOutput=False)
    wm_ext = nc.declare_dram_parameter("wm", [C, C], FP8, isOutput=False)
    wu_ext = nc.declare_dram_parameter("wu", [C, C], FP8, isOutput=False)
    gmat_ext = nc.declare_dram_parameter("gmat", [128, 128], BF16, isOutput=False)
    ones_ext = nc.declare_dram_parameter("ones", [128, 256], FP8, isOutput=False)
    out_ext = nc.declare_dram_parameter("out", [BPC, C, N], BF16, isOutput=True)

    with tile.TileContext(nc) as tc:
        with (
            tc.tile_pool(name="wpool", bufs=1) as wpool,
            tc.tile_pool(name="xpool", bufs=1) as xpool,
            tc.tile_pool(name="hpool", bufs=1) as hpool,
            tc.tile_pool(name="gpool", bufs=1) as gpool,
            tc.tile_pool(name="vepool", bufs=1) as vepool,
            tc.tile_pool(name="avpool", bufs=1) as avpool,
            tc.tile_pool(name="opool", bufs=4) as opool,
            tc.tile_pool(name="stpool", bufs=2) as stpool,
            tc.tile_pool(name="ps1", bufs=4, space="PSUM") as ps1,   # [128,512]
            tc.tile_pool(name="ps2", bufs=2, space="PSUM") as ps2,   # [128,1024]
        ):
            eps_sb = wpool.tile([128, 1], F32)
            nc.vector.memset(eps_sb, EPS)
            nln16_sb = wpool.tile([128, 1], F32)
            nc.vector.memset(nln16_sb, -LN16)

            # ---- input DMAs, first-use order, alternating queues ----
            xr = [x_ext[b].rearrange("(ko p) n -> p ko n", p=128) for b in range(BPC)]
            x0c0h = [
                xpool.tile([128, 512], BF16, name=f"x0c0h{i}", bufs=1) for i in range(2)
            ]
            x0_chunks = [None] + [
                xpool.tile([128, N], BF16, name=f"x0_{ki}", bufs=1)
                for ki in range(1, KT)
            ]
            nc.sync.dma_start(out=x0c0h[0], in_=xr[0][:, 0, 0:512])
            nc.gpsimd.dma_start(out=x0c0h[1], in_=xr[0][:, 0, 512:1024])
            nc.sync.dma_start(out=x0_chunks[1], in_=xr[0][:, 1, :])
            nc.gpsimd.dma_start(out=x0_chunks[2], in_=xr[0][:, 2, :])
            nc.sync.dma_start(out=x0_chunks[3], in_=xr[0][:, 3, :])
            gmat = wpool.tile([128, 128], BF16)
            nc.gpsimd.dma_start(out=gmat, in_=gmat_ext[:])
            wm = wpool.tile([128, KT, C], FP8)
            nc.sync.dma_start(out=wm, in_=wm_ext[:].rearrange("(ko p) f -> p ko f", p=128))
            wu = wpool.tile([128, KT, C], FP8)
            nc.gpsimd.dma_start(out=wu, in_=wu_ext[:].rearrange("(ko p) f -> p ko f", p=128))
            ones = wpool.tile([128, 256], FP8)
            nc.sync.dma_start(out=ones, in_=ones_ext[:])

            x1_pairs = [
                xpool.tile([128, 2, N], BF16, name=f"x1_{pr}", bufs=1) for pr in range(2)
            ]

            def x_chunk(b, ki):
                if b == 0:
                    if ki == 0:
                        return None  # halves; handled by callers
                    return x0_chunks[ki]
                return x1_pairs[ki // 2][:, ki % 2, :]

            # dummy ln: forces the single (ln+exp) ACT table load at t~7us
            warm = stpool.tile([128, 1], F32, name="warm")
            nc.scalar.activation(out=warm, in_=eps_sb, func=AF.Ln)

            h_prs = [
                [hpool.tile([128, 2, N], FP8, name=f"h_{b}_{pr}", bufs=1) for pr in range(2)]
                for b in range(BPC)
            ]
            g_sbs = [gpool.tile([128, KT, N], FP8, name=f"g_{b}", bufs=1) for b in range(BPC)]
            vTs = [vepool.tile([128, NT, C], FP8, name=f"vT_{b}", bufs=1) for b in range(BPC)]
            eTs = [vepool.tile([128, NT, N], FP8, name=f"eT_{b}", bufs=1) for b in range(BPC)]
            recips = [avpool.tile([128, N], F32, name=f"recip_{b}", bufs=1) for b in range(BPC)]
            # ACT accum scratch (full-width mandatory output of accum ops)
            scr = wpool.tile([128, N], F32)

            # per-batch stats: mv[:, ki, 0]=mean, mv[:, ki, 1]=E[x^2] (or var
            # pre-conversion for bn_stats chunks)
            mvs = [stpool.tile([128, KT, 2], F32, name=f"mv{b}", bufs=1) for b in range(BPC)]
            scls = [stpool.tile([128, KT], F32, name=f"scl{b}", bufs=1) for b in range(BPC)]
            sfts = [stpool.tile([128, KT], F32, name=f"sft{b}", bufs=1) for b in range(BPC)]

            def bn_chunk(b, ki, in0, in1):
                # DVE bn_stats path -> mv[:, ki, :] = (mean, var)
                mv = mvs[b]
                stats = stpool.tile([128, 2, 6], F32, name="stats")
                nc.vector.bn_stats(out=stats[:, 0, :], in_=in0)
                nc.vector.bn_stats(out=stats[:, 1, :], in_=in1)
                nc.vector.bn_aggr(out=mv[:, ki, :], in_=stats)

            def act_sum(b, ki, xc):
                # ACT accum: mean via Identity with scale 1/N
                nc.scalar.activation(
                    out=scr, in_=xc, func=AF.Identity, scale=1.0 / N,
                    accum_out=mvs[b][:, ki, 0:1],
                )

            def act_sq(b, ki, xc):
                # ACT accum: E[x^2] via Square with scale 1/32
                nc.scalar.activation(
                    out=scr, in_=xc, func=AF.Square, scale=1.0 / 32.0,
                    accum_out=mvs[b][:, ki, 1:2],
                )

            def e2_convert(b, ki):
                # var -> E[x^2] in place
                mv = mvs[b]
                msq = stpool.tile([128, 1], F32, name="msq")
                nc.vector.tensor_tensor(msq, mv[:, ki, 0:1], mv[:, ki, 0:1], OP.mult)
                nc.vector.tensor_tensor(mv[:, ki, 1:2], mv[:, ki, 1:2], msq, OP.add)

            def chain_pair(b, p):
                # group-reduce (mean, E2) for chunks 2p,2p+1 via gmat matmul,
                # then scl/sft for the pair.
                mv = mvs[b]
                mv_bf = stpool.tile([128, 4], BF16, name=f"mvbf{b}{p}", bufs=1)
                nc.vector.tensor_copy(
                    out=mv_bf, in_=mv[:, 2 * p : 2 * p + 2, :].rearrange("p a b -> p (a b)")
                )
                gn_ps = ps1.tile([128, 512], F32, name=f"gnps{b}{p}", tag="ps1")
                nc.tensor.matmul(gn_ps[:, 0:4], lhsT=gmat, rhs=mv_bf, start=True, stop=True)
                gs = stpool.tile([128, 4], F32, name=f"gs{b}{p}", bufs=1)
                nc.vector.tensor_copy(out=gs, in_=gn_ps[:, 0:4])
                gmean = gs[:, 0:4:2]
                gex2 = gs[:, 1:4:2]
                gvar = stpool.tile([128, 2], F32, name=f"gvar{b}{p}", bufs=1)
                nc.vector.tensor_tensor(gvar, gmean, gmean, OP.mult)
                nc.vector.tensor_tensor(gvar, gex2, gvar, OP.subtract)
                lnv = stpool.tile([128, 2], F32, name=f"lnv{b}{p}", bufs=1)
                nc.scalar.activation(out=lnv, in_=gvar, func=AF.Ln, bias=eps_sb)
                # rstd == scl (gamma==1, beta==0 guarded on host)
                scl_p = scls[b][:, 2 * p : 2 * p + 2]
                sft_p = sfts[b][:, 2 * p : 2 * p + 2]
                rstd = stpool.tile([128, 2], F32, name=f"rstd{b}{p}", bufs=1)
                nc.scalar.activation(out=rstd, in_=lnv, func=AF.Exp, scale=-0.5)
                nc.vector.tensor_copy(out=scl_p, in_=rstd)
                mscl = stpool.tile([128, 2], F32, name=f"mscl{b}{p}", bufs=1)
                nc.vector.tensor_tensor(mscl, gmean, scl_p, OP.mult)
                nc.vector.tensor_scalar_mul(sft_p, mscl, -1.0)
                return mv_bf

            def h_op(b, ki, eng, src, dst):
                eng.tensor_scalar(
                    out=dst, in0=src,
                    scalar1=scls[b][:, ki : ki + 1], scalar2=sfts[b][:, ki : ki + 1],
                    op0=OP.mult, op1=OP.add,
                )

            # ================= b0 head =================
            bn_chunk(0, 0, x0c0h[0], x0c0h[1])
            act_sum(0, 1, x0_chunks[1])
            act_sq(0, 1, x0_chunks[1])
            bn_chunk(0, 2, x0_chunks[2][:, 0:512], x0_chunks[2][:, 512:1024])
            e2_convert(0, 0)
            mvbf00 = chain_pair(0, 0)
            # gate b1's x DMA on an early b0-chain token: late enough that
            # the scheduler cannot hoist b1's work over b0's critical chain,
            # early enough that x1 lands while ACT still has stats headroom
            for pr in range(2):
                nc.gpsimd.tensor_scalar_add(
                    x1_pairs[pr][:, 0, 0:1], mvbf00[:, 0:1], 0.0
                )
            nc.sync.dma_start(out=x1_pairs[0], in_=xr[1][:, 0:2, :])
            nc.gpsimd.dma_start(out=x1_pairs[1], in_=xr[1][:, 2:4, :])
            act_sum(0, 3, x0_chunks[3])
            act_sq(0, 3, x0_chunks[3])
            e2_convert(0, 2)
            chain_pair(0, 1)

            # h(0): ki0 (halves) + ki2 on DVE, ki1/ki3 on GPSIMD
            h_op(0, 0, nc.vector, x0c0h[0], h_prs[0][0][:, 0, 0:512])
            h_op(0, 0, nc.vector, x0c0h[1], h_prs[0][0][:, 0, 512:1024])
            h_op(0, 1, nc.gpsimd, x0_chunks[1], h_prs[0][0][:, 1, :])
            h_op(0, 2, nc.vector, x0_chunks[2], h_prs[0][1][:, 0, :])
            h_op(0, 3, nc.gpsimd, x0_chunks[3], h_prs[0][1][:, 1, :])

            # ================= g(0) / vt(0): half-tiles in PS1 =================
            # g half-tile (oi, ni): [128,512] accumulating kk0,kk1
            # vt half-tile (nn, sub) likewise
            def g_mm(b, oi, ni, kk, ps):
                nc.tensor.matmul(
                    ps,
                    lhsT=wm[:, 2 * kk : 2 * kk + 2, oi * 128 : (oi + 1) * 128],
                    rhs=h_prs[b][kk][:, :, ni * 512 : (ni + 1) * 512],
                    start=(kk == 0), stop=(kk == 1), perf_mode=DR,
                )

            def vt_mm(b, ni, kk, ps):
                nc.tensor.matmul(
                    ps,
                    lhsT=h_prs[b][kk][:, :, ni * 128 : (ni + 1) * 128],
                    rhs=wu[:, 2 * kk : 2 * kk + 2, :],
                    start=(kk == 0), stop=(kk == 1), perf_mode=DR,
                )

            def g_drain(b, oi, ni, ps, eng):
                dst = g_sbs[b][:, oi, ni * 512 : (ni + 1) * 512]
                if eng is nc.scalar:
                    nc.scalar.activation(out=dst, in_=ps, func=AF.Identity)
                else:
                    eng.tensor_copy(out=dst, in_=ps)

            def vt_drain(b, ni, ps, eng):
                dst = vTs[b][:, ni, :]
                if eng is nc.scalar:
                    nc.scalar.activation(out=dst, in_=ps, func=AF.Identity)
                else:
                    eng.tensor_copy(out=dst, in_=ps)

            def emit_g(b, interleave_kk, acts):
                # 8 g half-tiles (oi, ni), 2 MMs each (kk0, kk1); `acts` of
                # the drains go to ACT, rest DVE
                keys = [(oi, ni) for oi in range(KT) for ni in range(2)]
                if interleave_kk:
                    # kk0s of 3 tiles in flight so pair1's late h never
                    # bubbles the PE queue
                    pipe = []
                    for idx, (oi, ni) in enumerate(keys):
                        ps = ps1.tile([128, 512], F32, name="gvt", tag="ps1")
                        g_mm(b, oi, ni, 0, ps)
                        pipe.append((oi, ni, ps))
                        if idx >= 3:
                            o2, n2, ps2_ = pipe.pop(0)
                            g_mm(b, o2, n2, 1, ps2_)
                            g_drain(b, o2, n2, ps2_,
                                    nc.scalar if (o2 * 2 + n2) < acts else nc.vector)
                    while pipe:
                        o2, n2, ps2_ = pipe.pop(0)
                        g_mm(b, o2, n2, 1, ps2_)
                        g_drain(b, o2, n2, ps2_,
                                nc.scalar if (o2 * 2 + n2) < acts else nc.vector)
                else:
                    for idx, (oi, ni) in enumerate(keys):
                        ps = ps1.tile([128, 512], F32, name="gvt", tag="ps1")
                        g_mm(b, oi, ni, 0, ps)
                        g_mm(b, oi, ni, 1, ps)
                        g_drain(b, oi, ni, ps,
                                nc.scalar if idx % 2 == 0 and idx < 2 * acts else nc.vector)

            def emit_vt(b, acts):
                for ni in range(NT):
                    ps = ps1.tile([128, 512], F32, name="gvt", tag="ps1")
                    vt_mm(b, ni, 0, ps)
                    vt_mm(b, ni, 1, ps)
                    vt_drain(b, ni, ps,
                             nc.scalar if ni % 2 == 0 and ni < 2 * acts else nc.vector)

            # b1 stats ride the otherwise-idle ACT window between b0's
            # chain and the b0 exp stream (accum -> (mean, E2) directly)
            for ki in range(KT):
                act_sum(1, ki, x_chunk(1, ki))
                act_sq(1, ki, x_chunk(1, ki))

            emit_g(0, interleave_kk=True, acts=2)
            emit_vt(0, acts=0)

            # ================= scores =================
            def emit_scores(b, mid=None):
                # eT = exp(g^T h * SCALE/WS - ln16)  [j, i]; denominators in
                # PS1 halves; recip on DVE. `mid` emits extra work after ji4.
                h_pr = h_prs[b]
                g_sb = g_sbs[b]
                eT = eTs[b]
                den_h = [ps1.tile([128, 512], F32, name=f"den{b}{i}", tag="ps1") for i in range(2)]

                def denom_mm(jj):
                    for ni in range(2):
                        nc.tensor.matmul(
                            den_h[ni],
                            lhsT=ones.rearrange("p (two f) -> p two f", two=2),
                            rhs=eT[:, 2 * jj : 2 * jj + 2, ni * 512 : (ni + 1) * 512],
                            start=(jj == 0), stop=(jj == NT // 2 - 1),
                            perf_mode=DR,
                        )

                for ji in range(NT):
                    ps = ps2.tile([128, N], F32, name="scps", tag="ps2")
                    for kk in range(2):
                        for ni in range(2):
                            nc.tensor.matmul(
                                ps[:, ni * 512 : (ni + 1) * 512],
                                lhsT=g_sb[:, 2 * kk : 2 * kk + 2, ji * 128 : (ji + 1) * 128],
                                rhs=h_pr[kk][:, :, ni * 512 : (ni + 1) * 512],
                                start=(kk == 0), stop=(kk == 1),
                                perf_mode=DR,
                            )
                    nc.scalar.activation(
                        out=eT[:, ji, :], in_=ps, func=AF.Exp,
                        bias=nln16_sb, scale=float(SCALE / WS),
                    )
                    if ji >= 3 and ji % 2 == 1:
                        denom_mm((ji - 3) // 2)
                    # mid at ji==2: early enough that b1's h is ready before
                    # g(1), late enough that the tiny gstat matmuls are
                    # data-ready when the in-order PE queue reaches them
                    if ji == 2 and mid is not None:
                        mid()
                denom_mm(NT // 2 - 1)
                recip = recips[b]
                nc.vector.reciprocal_approx_fast(out=recip[:, 0:512], in_=den_h[0])
                nc.vector.reciprocal_approx_fast(out=recip[:, 512:1024], in_=den_h[1])

            def b1_mid():
                # b1's GN chain + h, overlapped with b0's exp stream (the
                # ln/rstd ACT ops share the resident table set)
                chain_pair(1, 0)
                chain_pair(1, 1)
                x1c = [x_chunk(1, ki) for ki in range(KT)]
                h_op(1, 0, nc.vector, x1c[0], h_prs[1][0][:, 0, :])
                h_op(1, 1, nc.gpsimd, x1c[1], h_prs[1][0][:, 1, :])
                h_op(1, 2, nc.vector, x1c[2], h_prs[1][1][:, 0, :])
                h_op(1, 3, nc.gpsimd, x1c[3], h_prs[1][1][:, 1, :])

            emit_scores(0, mid=b1_mid)
            emit_g(1, interleave_kk=False, acts=2)
            emit_vt(1, acts=2)
            emit_scores(1)

            # ================= av =================
            def emit_av(b):
                vT = vTs[b]
                eT = eTs[b]
                recip = recips[b]
                o_r = out_ext[b].rearrange("(ko p) n -> p ko n", p=128)
                for ci in range(KT):
                    last = b == 1 and ci == KT - 1
                    if not last:
                        ps = ps2.tile([128, N], F32, name="avps", tag="ps2")
                        for jj in range(NT // 2):
                            for ni in range(2):
                                nc.tensor.matmul(
                                    ps[:, ni * 512 : (ni + 1) * 512],
                                    lhsT=vT[:, 2 * jj : 2 * jj + 2, ci * 128 : (ci + 1) * 128],
                                    rhs=eT[:, 2 * jj : 2 * jj + 2, ni * 512 : (ni + 1) * 512],
                                    start=(jj == 0), stop=(jj == NT // 2 - 1),
                                    perf_mode=DR,
                                )
                        t1 = opool.tile([128, N], BF16, name="t1_sb")
                        o_sb = opool.tile([128, N], BF16, name="o_sb")
                        nc.vector.tensor_tensor(t1, ps, recip, OP.mult)
                        # +x residual: gpsimd for early tiles (idle engine),
                        # DVE for late ones (gpsimd is slow, would drag tail)
                        add_eng = nc.gpsimd if ci < 2 else nc.vector
                        if b == 0 and ci == 0:
                            add_eng.tensor_tensor(o_sb[:, 0:512], t1[:, 0:512], x0c0h[0], OP.add)
                            add_eng.tensor_tensor(o_sb[:, 512:1024], t1[:, 512:1024], x0c0h[1], OP.add)
                        else:
                            add_eng.tensor_tensor(o_sb, t1, x_chunk(b, ci), OP.add)
                        eng = nc.sync if ci % 2 == 0 else nc.gpsimd
                        eng.dma_start(out=o_r[:, ci, :], in_=o_sb)
                    else:
                        # final tile: ni-major so each 512-half drains the
                        # moment its accumulation closes
                        ph = [ps1.tile([128, 512], F32, name=f"avl{i}", tag="ps1") for i in range(2)]
                        for ni in range(2):
                            for jj in range(NT // 2):
                                nc.tensor.matmul(
                                    ph[ni],
                                    lhsT=vT[:, 2 * jj : 2 * jj + 2, ci * 128 : (ci + 1) * 128],
                                    rhs=eT[:, 2 * jj : 2 * jj + 2, ni * 512 : (ni + 1) * 512],
                                    start=(jj == 0), stop=(jj == NT // 2 - 1),
                                    perf_mode=DR,
                                )
                            t1 = opool.tile([128, 512], BF16, name="t1h_sb")
                            o_sb = opool.tile([128, 512], BF16, name="oh_sb")
                            sl = slice(ni * 512, (ni + 1) * 512)
                            nc.vector.tensor_tensor(t1, ph[ni], recip[:, sl], OP.mult)
                            nc.vector.tensor_tensor(o_sb, t1, x_chunk(b, ci)[:, sl], OP.add)
                            eng = nc.gpsimd if ni == 0 else nc.sync
                            eng.dma_start(out=o_r[:, ci, sl], in_=o_sb)

            emit_av(0)
            emit_av(1)

    nc.compile()
    return nc


_NC_CACHE = None


def _get_nc():
    global _NC_CACHE
    if _NC_CACHE is None:
        _NC_CACHE = build_nc()
    return _NC_CACHE


def _prep_consts(gamma, beta, w_qkv, b_qkv, w_proj, b_proj):
    bf = ml_dtypes.bfloat16
    f8 = ml_dtypes.float8_e4m3
    w_q, w_k, w_v = w_qkv[0:C], w_qkv[C : 2 * C], w_qkv[2 * C : 3 * C]
    b_v = b_qkv[2 * C : 3 * C]
    m = w_q.astype(np.float64).T @ w_k.astype(np.float64)  # [C, C]
    wm = np.ascontiguousarray(m.T * WS).astype(f8)         # lhsT layout [b, a]
    u = w_proj.astype(np.float64) @ w_v.astype(np.float64)  # [C, C] proj-folded V
    wu = np.ascontiguousarray(u.T * WS).astype(f8)
    b_eff = w_proj.astype(np.float64) @ b_v.astype(np.float64) + b_proj
    gmat = (np.kron(np.eye(8, dtype=np.float32), np.ones((16, 16), np.float32)) / 16.0).astype(bf)
    # denominator lhsT: value WS compensates vT carrying a factor of WS
    ones = np.full((128, 256), WS, f8)
    return wm, wu, b_eff.astype(np.float32), gmat, ones


def make_in_maps(x, gamma, beta, w_qkv, b_qkv, w_proj, b_proj):
    bf = ml_dtypes.bfloat16
    x = np.asarray(x, np.float32)
    gamma = np.asarray(gamma, np.float32)
    beta = np.asarray(beta, np.float32)
    w_qkv = np.asarray(w_qkv, np.float32)
    b_qkv = np.asarray(b_qkv, np.float32)
    w_proj = np.asarray(w_proj, np.float32)
    b_proj = np.asarray(b_proj, np.float32)
    wm, wu, b_eff, gmat, ones = _prep_consts(
        gamma, beta, w_qkv, b_qkv, w_proj, b_proj
    )
    # b_eff rides the residual input: GroupNorm is invariant to a per-channel
    # shift (the mean absorbs it), and softmax rows sum to 1, so shipping
    # x + b_eff makes out = (x + b_eff) + U h A exactly the reference result.
    xr = np.ascontiguousarray(
        (x.reshape(B, C, N) + b_eff[None, :, None]).astype(bf)
    )
    return [
        {
            "x": xr[i * BPC : (i + 1) * BPC],
            "wm": wm,
            "wu": wu,
            "gmat": gmat,
            "ones": ones,
        }
        for i in range(N_CORES)
    ]


def _numpy_fallback(x, gamma, beta, w_qkv, b_qkv, w_proj, b_proj):
    # Exact reference implementation; only used when b_q is nonzero (the
    # device graph folds Wq^T Wk and drops the q-bias term, which is exact
    # for this model where b_qkv == 0).
    Bs, Cs, Hs, Ws_ = x.shape
    g = x.reshape(Bs, GROUPS, Cs // GROUPS, Hs, Ws_)
    mu = g.mean(axis=(2, 3, 4), keepdims=True)
    var = g.var(axis=(2, 3, 4), keepdims=True)
    g = (g - mu) / np.sqrt(var + EPS)
    h = g.reshape(Bs, Cs, Hs, Ws_) * gamma[None, :, None, None] + beta[None, :, None, None]
    hn = h.reshape(Bs, Cs, N)
    qkv = np.einsum("bcn,oc->bon", hn, w_qkv) + b_qkv[None, :, None]
    q, k, v = qkv[:, :Cs], qkv[:, Cs : 2 * Cs], qkv[:, 2 * Cs :]
    s = np.einsum("bci,bcj->bij", q, k) / np.sqrt(np.float32(Cs))
    s = s - s.max(axis=-1, keepdims=True)
    e = np.exp(s)
    a = e / e.sum(axis=-1, keepdims=True)
    o = np.einsum("bij,bcj->bci", a, v)
    o = np.einsum("bcn,oc->bon", o, w_proj) + b_proj[None, :, None]
    return (x + o.reshape(Bs, Cs, Hs, Ws_)).astype(np.float32)


def kernel(x, gamma, beta, w_qkv, b_qkv, w_proj, b_proj):
    from concourse.bass_utils import run_bass_kernel_spmd

    x = np.asarray(x, np.float32)
    b_qkv = np.asarray(b_qkv, np.float32)
    gamma_a = np.asarray(gamma, np.float32)
    beta_a = np.asarray(beta, np.float32)
    # the device graph hardcodes gamma==1 / beta==0 (true for this model);
    # exact numpy path otherwise
    if (
        np.abs(b_qkv[0:C]).max() > 1e-7
        or np.abs(gamma_a - 1.0).max() > 1e-7
        or np.abs(beta_a).max() > 1e-7
    ):
        return _numpy_fallback(
            x, np.asarray(gamma, np.float32), np.asarray(beta, np.float32),
            np.asarray(w_qkv, np.float32), b_qkv,
            np.asarray(w_proj, np.float32), np.asarray(b_proj, np.float32),
        )

    nc = _get_nc()
    in_maps = make_in_maps(x, gamma, beta, w_qkv, b_qkv, w_proj, b_proj)
    res = run_bass_kernel_spmd(nc, in_maps, core_ids=list(range(N_CORES)))
    out = np.concatenate([res.results[i]["out"] for i in range(N_CORES)], axis=0)
    return np.ascontiguousarray(out.reshape(B, C, H, W), dtype=np.float32)
